# revision 17
# baseline (speedup 1.0000x reference)
"""Trainium2 Bass kernel for DigitConvolutionalModel.

Reference computation (B = 32768):
    x: [B, 784] -> reshape [B, 28, 28]
    conv 3x3 valid with w_conv -> [B, 26, 26] -> [B, 676]
    h1 = relu(conv @ W1 + b1)    W1: [676, 100]
    h2 = relu(h1 @ W2 + b2)      W2: [100, 100]
    out = h2 @ W3 + b3           W3: [100, 10]

Strategy
--------
Pure data parallel: batch split 8 ways (4096 rows/core), weights replicated.
The conv is linear, so it is folded into W1 on the host:
    conv(x) @ W1 == x @ (M @ W1) = x @ W1e,  W1e: [784, 100]
removing the conv from the device entirely (exact up to fp rounding).

On-device layout is "transposed": features on SBUF partitions, batch on the
free dimension, so each layer's PSUM output feeds the next matmul directly
as the moving operand. The host pre-transposes x per core and lays it out
as [128, 6, B_LOC] (contraction split 784 = 6*128 + 16; the 16-row tail
rides a [16, 128 + B_LOC] tile with the matching W1e rows) so every x DMA
uses all 128 partitions with long contiguous runs.

x and the weights are cast to fp16 on the host: fp16's 10-bit mantissa
keeps end-to-end error at ~6e-4 relative (measured) while halving HBM
traffic and running every matmul at full PE rate. The kernel is
HBM-bandwidth bound streaming x (~6.4 MB/core).

Trace-analysis findings this version is built on:
- FWL (fast weight load) requires exactly 128 weight columns, so all
  weight matrices are zero-padded to 128 columns; this cut the measured
  warm matmul cadence from ~330ns to ~236ns at N=512.
- The PE HAM clock gate needs ~3.4us of sustained NONZERO matmul work to
  reach 2.4 GHz (zero-valued warmups do not register). ~30 N=128 warmup
  matmuls on a ones-tile bridge the preamble-to-first-data window.
- Only 8 HWDGE completion-sem lanes exist across both HWDGE rings; DMA
  issue k+8 waits for DMA k's completion. All weights+biases are merged
  into ONE early DMA (biases as fp16 columns), the tail-row tile rides
  the sync ring behind group 0, and output stores go through gpsimd
  SWDGE (its own 8 lanes) so the x stream owns the HWDGE lanes.
- The 16-row contraction tail matmuls are emitted after each group's main
  chunks (carrying the accumulation stop flag) so no mid-group data
  dependency on the tail tile exists.
"""

import numpy as np

N_CORES = 8
B = 32768
B_LOC = B // N_CORES          # 4096 rows per core
NT = 512                      # matmul moving-dim tile (PSUM bank limit)
GROUPS = [2048, 1024, 256, 256, 256]  # emission: g0,g1,g4(c0-c4),g2,g3,g4c5
KC = 6                        # full 128-row contraction chunks
KT = 784 - KC * 128           # 16-row tail
H = 128                       # hidden width, zero-padded 100 -> 128 (FWL)
O = 10                        # output width
WARMUP_MMS = 44               # dummy N=128 matmuls to warm the PE clock gate
                              # (must exceed the 3413ns HAM activity window)
N_PS1 = 5                     # rotating layer-1 PSUM accumulator banks
WCOLS = KC * H + 2 * H        # W1e chunks | W2 | W3

_COMPILED = {}
LAST_RESULTS = None


def _build_nc():
    import concourse.mybir as mybir
    from concourse import bacc
    from concourse.tile import TileContext

    f32 = mybir.dt.float32
    f16 = mybir.dt.float16

    nc = bacc.Bacc(
        "TRN2", target_bir_lowering=False, debug=False, num_devices=N_CORES
    )
    xt = nc.dram_tensor("xt", [128, KC, B_LOC], f16, kind="ExternalInput")
    wt = nc.dram_tensor("wt", [128, WCOLS], f16, kind="ExternalInput")
    # packed [16, 128 + B_LOC]: W1e tail rows | x tail rows
    wxl = nc.dram_tensor("wxl", [KT, H + B_LOC], f16, kind="ExternalInput")
    # packed [128, 3]: b1 | b2 | b3 (b3 on partitions 0..9) - rides SWDGE
    bb = nc.dram_tensor("bb", [H, 3], f32, kind="ExternalInput")
    ot = nc.dram_tensor("ot", [O, B_LOC], f32, kind="ExternalOutput")

    relu = mybir.ActivationFunctionType.Relu
    add = mybir.AluOpType.add
    amax = mybir.AluOpType.max

    with TileContext(nc) as tc:
        with (
            tc.tile_pool(name="wpool", bufs=1) as wpool,
            tc.tile_pool(name="xpool", bufs=1) as xpool,
            tc.tile_pool(name="hpool", bufs=3) as hpool,
            tc.tile_pool(name="opool", bufs=2) as opool,
            tc.tile_pool(name="ppool", bufs=1, space="PSUM") as ppool,
        ):
            # Warmup matmuls on a ones-tile: their only dep is a tiny
            # memset, so the PE runs them right after the preamble barrier
            # while the first x chunks stream in, flipping the HAM clock
            # gate to 2.4 GHz before real work arrives. Results land in
            # the ps2 bank (first real use is ~10us later), never read.
            warm_t = wpool.tile([128, 128], f16)
            nc.gpsimd.memset(warm_t, 1.0)
            # biases ride the SWDGE path: gpsimd is idle early and its
            # completion lanes are separate from the 8 HWDGE lanes
            bb_t = wpool.tile([H, 3], f32)
            nc.gpsimd.dma_start(out=bb_t, in_=bb.ap())
            ps_w = ppool.tile([128, NT], f32, tag="ps2", bufs=2, name="ps_w")
            for _ in range(WARMUP_MMS):
                nc.tensor.matmul(
                    ps_w[:, :128], lhsT=warm_t, rhs=warm_t,
                    start=True, stop=True,
                )

            # single early weight+bias DMA on the scalar ring; x owns sync
            wt_t = wpool.tile([128, WCOLS], f16)
            nc.scalar.dma_start(out=wt_t, in_=wt.ap())
            w1_t = wt_t[:, 0 : KC * H]
            w2_t = wt_t[:, KC * H : KC * H + H]
            w3_t = wt_t[:, KC * H + H : KC * H + 2 * H]
            b1_t = bb_t[:, 0:1]
            b2_t = bb_t[:, 1:2]
            b3_t = bb_t[:O, 2:3]

            wxl_t = wpool.tile([KT, H + B_LOC], f16)
            w1l_t = wxl_t[:, 0:H]
            xl_t = wxl_t[:, H : H + B_LOC]

            def epilogue(g0, ntd, subt, nt, ps1s):
                # stage-major across subtiles so the per-engine FIFOs don't
                # head-of-line block the chains; relu on ACT, everything
                # else element-wise on DVE; one merged output store on the
                # SWDGE path (own completion lanes, frees HWDGE for x)
                h1s, h2s = [], []
                o_t = opool.tile(
                    [O, ntd], f32, tag="o_t", bufs=2, name=f"o_{g0}"
                )
                for s in range(subt):
                    h1 = hpool.tile([H, NT], f16, tag="h1", bufs=4, name=f"h1_{s}")
                    nc.scalar.activation(
                        h1[:, :nt], ps1s[s][:, :nt], relu, bias=b1_t
                    )
                    h1s.append(h1)
                for s in range(subt):
                    ps2 = ppool.tile([128, NT], f32, tag="ps2", bufs=2, name="ps2")
                    nc.tensor.matmul(
                        ps2[:, :nt], lhsT=w2_t, rhs=h1s[s][:, :nt],
                        start=True, stop=True,
                    )
                    h2 = hpool.tile([H, NT], f16, tag="h2", bufs=4, name=f"h2_{s}")
                    nc.vector.tensor_scalar(
                        h2[:, :nt], ps2[:, :nt], b2_t, 0.0, add, amax
                    )
                    h2s.append(h2)
                for s in range(subt):
                    ps3 = ppool.tile([128, NT], f32, tag="ps3", bufs=1, name="ps3")
                    nc.tensor.matmul(
                        ps3[:, :nt], lhsT=w3_t, rhs=h2s[s][:, :nt],
                        start=True, stop=True,
                    )
                    nc.vector.tensor_scalar(
                        o_t[:, s * nt : (s + 1) * nt], ps3[:O, :nt],
                        b3_t, None, add,
                    )
                nc.gpsimd.dma_start(out=ot.ap()[:, g0 : g0 + ntd], in_=o_t)

            def epilogue_last(g0, nt, ps1):
                # the final epilogue is fully exposed after the x stream
                # ends; run it in two half-chains so the ACT/PE/DVE stages
                # pipeline and the serial latency halves. Stores go on the
                # sync HWDGE ring: no x DMAs remain to couple with and
                # HWDGE completion beats SWDGE to the final drain.
                NH = nt // 2
                h1 = hpool.tile([H, NT], f16, tag="h1", bufs=4, name="h1_l")
                h2 = hpool.tile([H, NT], f16, tag="h2", bufs=4, name="h2_l")
                o_t = opool.tile([O, NT], f32, tag="o_t", bufs=2, name="o_l")
                ps2 = ppool.tile([128, NT], f32, tag="ps1_0", bufs=1, name="ps2l")
                ps3 = ppool.tile([128, NT], f32, tag="ps1_4", bufs=1, name="ps3l")
                for hh in range(2):
                    cs = slice(hh * NH, (hh + 1) * NH)
                    nc.scalar.activation(h1[:, cs], ps1[:, cs], relu, bias=b1_t)
                    nc.tensor.matmul(
                        ps2[:, cs], lhsT=w2_t, rhs=h1[:, cs],
                        start=True, stop=True,
                    )
                    nc.vector.tensor_scalar(
                        h2[:, cs], ps2[:, cs], b2_t, 0.0, add, amax
                    )
                    nc.tensor.matmul(
                        ps3[:, cs], lhsT=w3_t, rhs=h2[:, cs],
                        start=True, stop=True,
                    )
                    nc.vector.tensor_scalar(
                        o_t[:, cs], ps3[:O, cs], b3_t, None, add
                    )
                    nc.sync.dma_start(
                        out=ot.ap()[:, g0 + hh * NH : g0 + (hh + 1) * NH],
                        in_=o_t[:, cs],
                    )

            # x DMA plan: the stream order is chosen so the LAST bytes to
            # arrive are a single tiny chunk of the final 256-col group --
            # its other 5 chunks ship mid-stream and their matmuls run
            # early, so only one matmul plus the final chain are exposed
            # after the stream ends. Weights/biases ride separate paths.
            PLANS = {
                2048: [(0, 1), (1, 2), (3, 2), (5, 1)],
                1024: [(0, 2), (2, 2), (4, 2)],
                512: [(0, 2), (2, 2), (4, 2)],
                256: [(0, 2), (2, 2), (4, 2)],
            }
            G0S = [0, 2048, 3072, 3584, 3840]
            NTDS = [2048, 1024, 512, 256, 256]

            state = {"pending": None, "rot": 0}

            def alloc_ps1(subt):
                ps1s = [
                    ppool.tile(
                        [128, NT], f32,
                        tag=f"ps1_{(state['rot'] + s) % N_PS1}",
                        bufs=1, name=f"ps1_{s}",
                    )
                    for s in range(subt)
                ]
                state["rot"] += subt
                return ps1s

            def dma_group(g, plan_entries, ntd, gs):
                xc = []
                for c0, w in plan_entries:
                    x_c = xpool.tile(
                        [128, w, ntd], f16, tag=f"xc{g}_{c0}", bufs=1,
                        name=f"xc{g}_{c0}",
                    )
                    nc.sync.dma_start(out=x_c, in_=xt.ap()[:, c0 : c0 + w, gs])
                    xc.append((x_c, c0, w))
                return xc

            def mm_group(xc, ps1s, subt, nt, g0, tail_stop, first_c):
                done_c = 0
                for x_c, c0, w in xc:
                    for ci in range(w):
                        c = c0 + ci
                        for s in range(subt):
                            nc.tensor.matmul(
                                ps1s[s][:, :nt],
                                lhsT=w1_t[:, c * H : (c + 1) * H],
                                rhs=x_c[:, ci, s * nt : (s + 1) * nt],
                                start=(c == first_c),
                                stop=False,
                            )
                    done_c += w
                    if done_c >= 2 and state["pending"] is not None:
                        epilogue(*state["pending"])
                        state["pending"] = None
                if tail_stop is not None:
                    for s in range(subt):
                        nc.tensor.matmul(
                            ps1s[s][:, :nt],
                            lhsT=w1l_t,
                            rhs=xl_t[:, g0 + s * nt : g0 + (s + 1) * nt],
                            start=False,
                            stop=tail_stop,
                        )

            # group 0 (2048) and group 1 (1024): stream + matmul + epilogue
            infos = []
            for g in (0, 1):
                ntd = NTDS[g]
                nt = min(NT, ntd)
                subt = ntd // nt
                gs = slice(G0S[g], G0S[g] + ntd)
                xc = dma_group(g, PLANS[ntd], ntd, gs)
                if g == 0:
                    nc.sync.dma_start(out=wxl_t, in_=wxl.ap())
                ps1s = alloc_ps1(subt)
                mm_group(xc, ps1s, subt, nt, G0S[g], True, 0)
                state["pending"] = (G0S[g], ntd, subt, nt, ps1s)
                infos.append((ps1s, nt))

            # group 4 (the final 256): chunks 0..4 + tail mid-stream
            g4_ntd = NTDS[4]
            g4_nt = 256
            g4_gs = slice(G0S[4], G0S[4] + g4_ntd)
            g4_xc = dma_group(4, [(0, 2), (2, 2), (4, 1)], g4_ntd, g4_gs)
            g4_ps1 = alloc_ps1(1)
            mm_group(g4_xc, g4_ps1, 1, g4_nt, G0S[4], False, 0)

            # groups 2 and 3 (256 each)
            for g in (2, 3):
                ntd = NTDS[g]
                nt = min(NT, ntd)
                subt = ntd // nt
                gs = slice(G0S[g], G0S[g] + ntd)
                xc = dma_group(g, PLANS[ntd], ntd, gs)
                ps1s = alloc_ps1(subt)
                mm_group(xc, ps1s, subt, nt, G0S[g], True, 0)
                state["pending"] = (G0S[g], ntd, subt, nt, ps1s)

            # final chunk of group 4: the last bytes of the stream
            g4_c5 = xpool.tile(
                [128, 1, g4_ntd], f16, tag="xc4_5", bufs=1, name="xc4_5"
            )
            nc.sync.dma_start(out=g4_c5, in_=xt.ap()[:, 5:6, g4_gs])
            nc.tensor.matmul(
                g4_ps1[0][:, :g4_nt],
                lhsT=w1_t[:, 5 * H : 6 * H],
                rhs=g4_c5[:, 0, 0:g4_nt],
                start=False, stop=True,
            )
            if state["pending"] is not None:
                epilogue(*state["pending"])
                state["pending"] = None
            epilogue_last(G0S[4], g4_nt, g4_ps1[0])

    nc.finalize()
    return nc


def _fold_conv_into_w1(w_conv, W1):
    """W1e[784, 100] such that x @ W1e == conv3x3(x) @ W1 (exact linear fold)."""
    W1e = np.zeros((28, 28, 100), np.float64)
    W1r = W1.astype(np.float64).reshape(26, 26, 100)
    wc = w_conv.astype(np.float64)
    for di in range(3):
        for dj in range(3):
            W1e[di : di + 26, dj : dj + 26, :] += wc[di, dj] * W1r
    return W1e.reshape(784, 100).astype(np.float32)


def kernel(x, w_conv, W1, b1, W2, b2, W3, b3):
    from concourse.bass_utils import run_bass_kernel_spmd

    global LAST_RESULTS

    x = np.asarray(x, np.float32)
    W1e = _fold_conv_into_w1(np.asarray(w_conv), np.asarray(W1))
    W1p = np.zeros((784, H), np.float32)
    W1p[:, :100] = W1e
    # wt: [128, 6*128 | W2 | W3 | b1 b2 b3] all fp16
    wt_dev = np.zeros((128, WCOLS), np.float16)
    wt_dev[:, : KC * H] = (
        W1p[: KC * 128].reshape(KC, 128, H).transpose(1, 0, 2).reshape(128, KC * H)
    ).astype(np.float16)
    wt_dev[:100, KC * H : KC * H + 100] = (
        np.asarray(W2, np.float32).astype(np.float16)
    )
    wt_dev[:100, KC * H + H : KC * H + H + O] = (
        np.asarray(W3, np.float32).astype(np.float16)
    )
    w1l_dev = W1p[KC * 128 :].astype(np.float16)      # [16, 128]
    bb_dev = np.zeros((H, 3), np.float32)
    bb_dev[:100, 0] = np.asarray(b1, np.float32)
    bb_dev[:100, 1] = np.asarray(b2, np.float32)
    bb_dev[:O, 2] = np.asarray(b3, np.float32)

    in_maps = []
    for c in range(N_CORES):
        xs = x[c * B_LOC : (c + 1) * B_LOC]          # [B_LOC, 784]
        xT = xs.T.astype(np.float16)                  # [784, B_LOC] fp16
        # main: [128, KC, B_LOC], element [p, k, n] = xT[k*128 + p, n]
        xmain = np.ascontiguousarray(
            xT[: KC * 128].reshape(KC, 128, B_LOC).transpose(1, 0, 2)
        )
        wxl_dev = np.concatenate([w1l_dev, xT[KC * 128 :]], axis=1)
        in_maps.append(
            {
                "xt": xmain,
                "wxl": np.ascontiguousarray(wxl_dev),
                "wt": wt_dev,
                "bb": bb_dev,
            }
        )

    if "nc" not in _COMPILED:
        _COMPILED["nc"] = _build_nc()
    nc = _COMPILED["nc"]

    res = run_bass_kernel_spmd(nc, in_maps, core_ids=list(range(N_CORES)))
    LAST_RESULTS = res

    out = np.empty((B, O), np.float32)
    for c in range(N_CORES):
        out[c * B_LOC : (c + 1) * B_LOC] = res.results[c]["ot"].T
    return out


# revision 18
# speedup vs baseline: 1.1644x; 1.1644x over previous
"""Trainium2 Bass kernel for DigitConvolutionalModel.

Reference computation (B = 32768):
    x: [B, 784] -> reshape [B, 28, 28]
    conv 3x3 valid with w_conv -> [B, 26, 26] -> [B, 676]
    h1 = relu(conv @ W1 + b1)    W1: [676, 100]
    h2 = relu(h1 @ W2 + b2)      W2: [100, 100]
    out = h2 @ W3 + b3           W3: [100, 10]

Strategy
--------
Pure data parallel: batch split 8 ways (4096 rows/core), weights replicated.
The conv is linear, so it is folded into W1 on the host:
    conv(x) @ W1 == x @ (M @ W1) = x @ W1e,  W1e: [784, 100]
removing the conv from the device entirely (exact up to fp rounding).

On-device layout is "transposed": features on SBUF partitions, batch on the
free dimension, so each layer's PSUM output feeds the next matmul directly
as the moving operand. The host pre-transposes x per core and lays it out
as [128, 6, B_LOC] (contraction split 784 = 6*128 + 16; the 16-row tail
rides a [16, 128 + B_LOC] tile with the matching W1e rows) so every x DMA
uses all 128 partitions with long contiguous runs.

x and the weights are cast to fp16 on the host: fp16's 10-bit mantissa
keeps end-to-end error at ~6e-4 relative (measured) while halving HBM
traffic and running every matmul at full PE rate. The kernel is
HBM-bandwidth bound streaming x (~6.4 MB/core).

Trace-analysis findings this version is built on:
- FWL (fast weight load) requires exactly 128 weight columns, so all
  weight matrices are zero-padded to 128 columns; this cut the measured
  warm matmul cadence from ~330ns to ~236ns at N=512.
- The PE HAM clock gate needs ~3.4us of sustained NONZERO matmul work to
  reach 2.4 GHz (zero-valued warmups do not register). ~30 N=128 warmup
  matmuls on a ones-tile bridge the preamble-to-first-data window.
- Only 8 HWDGE completion-sem lanes exist across both HWDGE rings; DMA
  issue k+8 waits for DMA k's completion. All weights+biases are merged
  into ONE early DMA (biases as fp16 columns), the tail-row tile rides
  the sync ring behind group 0, and output stores go through gpsimd
  SWDGE (its own 8 lanes) so the x stream owns the HWDGE lanes.
- The 16-row contraction tail matmuls are emitted after each group's main
  chunks (carrying the accumulation stop flag) so no mid-group data
  dependency on the tail tile exists.
"""

import numpy as np

N_CORES = 8
B = 32768
B_LOC = B // N_CORES          # 4096 rows per core
NT = 512                      # matmul moving-dim tile (PSUM bank limit)
GROUPS = [2048, 1024, 256, 256, 256]  # emission: g0,g1,g4(c0-c4),g2,g3,g4c5
KC = 6                        # full 128-row contraction chunks
KT = 784 - KC * 128           # 16-row tail
H = 128                       # hidden width, zero-padded 100 -> 128 (FWL)
O = 10                        # output width
WARMUP_MMS = 44               # dummy N=128 matmuls to warm the PE clock gate
                              # (must exceed the 3413ns HAM activity window)
N_PS1 = 5                     # rotating layer-1 PSUM accumulator banks
WCOLS = KC * H + 2 * H        # W1e chunks | W2 | W3

_COMPILED = {}
LAST_RESULTS = None


def _build_nc():
    import concourse.mybir as mybir
    from concourse import bacc
    from concourse.tile import TileContext

    f32 = mybir.dt.float32
    f16 = mybir.dt.float16

    nc = bacc.Bacc(
        "TRN2", target_bir_lowering=False, debug=False, num_devices=N_CORES
    )
    xt = nc.dram_tensor("xt", [128, KC, B_LOC], f16, kind="ExternalInput")
    wt = nc.dram_tensor("wt", [128, WCOLS], f16, kind="ExternalInput")
    # packed [16, 128 + B_LOC]: W1e tail rows | x tail rows
    wxl = nc.dram_tensor("wxl", [KT, H + B_LOC], f16, kind="ExternalInput")
    # packed [128, 3]: b1 | b2 | b3 (b3 on partitions 0..9) - rides SWDGE
    bb = nc.dram_tensor("bb", [H, 3], f32, kind="ExternalInput")
    ot = nc.dram_tensor("ot", [O, B_LOC], f32, kind="ExternalOutput")

    relu = mybir.ActivationFunctionType.Relu
    add = mybir.AluOpType.add
    amax = mybir.AluOpType.max

    with TileContext(nc) as tc:
        with (
            tc.tile_pool(name="wpool", bufs=1) as wpool,
            tc.tile_pool(name="xpool", bufs=1) as xpool,
            tc.tile_pool(name="hpool", bufs=3) as hpool,
            tc.tile_pool(name="opool", bufs=2) as opool,
            tc.tile_pool(name="ppool", bufs=1, space="PSUM") as ppool,
        ):
            # Warmup matmuls on a ones-tile: their only dep is a tiny
            # memset, so the PE runs them right after the preamble barrier
            # while the first x chunks stream in, flipping the HAM clock
            # gate to 2.4 GHz before real work arrives. Results land in
            # the ps2 bank (first real use is ~10us later), never read.
            warm_t = wpool.tile([128, 128], f16)
            nc.gpsimd.memset(warm_t, 1.0)
            # biases ride the SWDGE path: gpsimd is idle early and its
            # completion lanes are separate from the 8 HWDGE lanes
            bb_t = wpool.tile([H, 3], f32)
            nc.gpsimd.dma_start(out=bb_t, in_=bb.ap())
            ps_w = ppool.tile([128, NT], f32, tag="ps2", bufs=2, name="ps_w")
            for _ in range(WARMUP_MMS):
                nc.tensor.matmul(
                    ps_w[:, :128], lhsT=warm_t, rhs=warm_t,
                    start=True, stop=True,
                )

            # single early weight+bias DMA on the scalar ring; x owns sync
            wt_t = wpool.tile([128, WCOLS], f16)
            nc.scalar.dma_start(out=wt_t, in_=wt.ap())
            w1_t = wt_t[:, 0 : KC * H]
            w2_t = wt_t[:, KC * H : KC * H + H]
            w3_t = wt_t[:, KC * H + H : KC * H + 2 * H]
            b1_t = bb_t[:, 0:1]
            b2_t = bb_t[:, 1:2]
            b3_t = bb_t[:O, 2:3]

            wxl_t = wpool.tile([KT, H + B_LOC], f16)
            w1l_t = wxl_t[:, 0:H]
            xl_t = wxl_t[:, H : H + B_LOC]

            def epilogue(g0, ntd, subt, nt, ps1s):
                # stage-major across subtiles so the per-engine FIFOs don't
                # head-of-line block the chains; relu on ACT, everything
                # else element-wise on DVE; one merged output store on the
                # SWDGE path (own completion lanes, frees HWDGE for x)
                h1s, h2s = [], []
                o_t = opool.tile(
                    [O, ntd], f32, tag="o_t", bufs=2, name=f"o_{g0}"
                )
                for s in range(subt):
                    h1 = hpool.tile([H, NT], f16, tag="h1", bufs=4, name=f"h1_{s}")
                    nc.scalar.activation(
                        h1[:, :nt], ps1s[s][:, :nt], relu, bias=b1_t
                    )
                    h1s.append(h1)
                for s in range(subt):
                    ps2 = ppool.tile([128, NT], f32, tag="ps2", bufs=2, name="ps2")
                    nc.tensor.matmul(
                        ps2[:, :nt], lhsT=w2_t, rhs=h1s[s][:, :nt],
                        start=True, stop=True,
                    )
                    h2 = hpool.tile([H, NT], f16, tag="h2", bufs=4, name=f"h2_{s}")
                    nc.vector.tensor_scalar(
                        h2[:, :nt], ps2[:, :nt], b2_t, 0.0, add, amax
                    )
                    h2s.append(h2)
                for s in range(subt):
                    ps3 = ppool.tile([128, NT], f32, tag="ps3", bufs=1, name="ps3")
                    nc.tensor.matmul(
                        ps3[:, :nt], lhsT=w3_t, rhs=h2s[s][:, :nt],
                        start=True, stop=True,
                    )
                    nc.vector.tensor_scalar(
                        o_t[:, s * nt : (s + 1) * nt], ps3[:O, :nt],
                        b3_t, None, add,
                    )
                nc.gpsimd.dma_start(out=ot.ap()[:, g0 : g0 + ntd], in_=o_t)

            def epilogue_last(g0, nt, ps1):
                # the final epilogue is fully exposed after the x stream
                # ends; run it in two half-chains so the ACT/PE/DVE stages
                # pipeline and the serial latency halves. Stores go on the
                # sync HWDGE ring: no x DMAs remain to couple with and
                # HWDGE completion beats SWDGE to the final drain.
                NH = nt // 2
                h1 = hpool.tile([H, NT], f16, tag="h1", bufs=4, name="h1_l")
                h2 = hpool.tile([H, NT], f16, tag="h2", bufs=4, name="h2_l")
                o_t = opool.tile([O, NT], f32, tag="o_t", bufs=2, name="o_l")
                ps2 = ppool.tile([128, NT], f32, tag="ps1_0", bufs=1, name="ps2l")
                ps3 = ppool.tile([128, NT], f32, tag="ps1_4", bufs=1, name="ps3l")
                for hh in range(2):
                    cs = slice(hh * NH, (hh + 1) * NH)
                    nc.scalar.activation(h1[:, cs], ps1[:, cs], relu, bias=b1_t)
                    nc.tensor.matmul(
                        ps2[:, cs], lhsT=w2_t, rhs=h1[:, cs],
                        start=True, stop=True,
                    )
                    nc.vector.tensor_scalar(
                        h2[:, cs], ps2[:, cs], b2_t, 0.0, add, amax
                    )
                    nc.tensor.matmul(
                        ps3[:, cs], lhsT=w3_t, rhs=h2[:, cs],
                        start=True, stop=True,
                    )
                    nc.vector.tensor_scalar(
                        o_t[:, cs], ps3[:O, cs], b3_t, None, add
                    )
                    nc.sync.dma_start(
                        out=ot.ap()[:, g0 + hh * NH : g0 + (hh + 1) * NH],
                        in_=o_t[:, cs],
                    )

            # x DMA plan: the stream order is chosen so the LAST bytes to
            # arrive are a single tiny chunk of the final 256-col group --
            # its other 5 chunks ship mid-stream and their matmuls run
            # early, so only one matmul plus the final chain are exposed
            # after the stream ends. Weights/biases ride separate paths.
            PLANS = {
                2048: [(0, 1), (1, 1), (2, 1), (3, 1), (4, 1), (5, 1)],
                1024: [(0, 2), (2, 2), (4, 2)],
                512: [(0, 2), (2, 2), (4, 2)],
                256: [(0, 2), (2, 2), (4, 2)],
            }
            G0S = [0, 2048, 3072, 3584, 3840]
            NTDS = [2048, 1024, 512, 256, 256]

            state = {"pending": None, "rot": 0}

            def alloc_ps1(subt):
                ps1s = [
                    ppool.tile(
                        [128, NT], f32,
                        tag=f"ps1_{(state['rot'] + s) % N_PS1}",
                        bufs=1, name=f"ps1_{s}",
                    )
                    for s in range(subt)
                ]
                state["rot"] += subt
                return ps1s

            def dma_group(g, plan_entries, ntd, gs):
                xc = []
                for c0, w in plan_entries:
                    x_c = xpool.tile(
                        [128, w, ntd], f16, tag=f"xc{g}_{c0}", bufs=1,
                        name=f"xc{g}_{c0}",
                    )
                    nc.sync.dma_start(out=x_c, in_=xt.ap()[:, c0 : c0 + w, gs])
                    xc.append((x_c, c0, w))
                return xc

            def mm_group(xc, ps1s, subt, nt, g0, tail_stop, first_c):
                done_c = 0
                for x_c, c0, w in xc:
                    for ci in range(w):
                        c = c0 + ci
                        for s in range(subt):
                            nc.tensor.matmul(
                                ps1s[s][:, :nt],
                                lhsT=w1_t[:, c * H : (c + 1) * H],
                                rhs=x_c[:, ci, s * nt : (s + 1) * nt],
                                start=(c == first_c),
                                stop=False,
                            )
                    done_c += w
                    if done_c >= 2 and state["pending"] is not None:
                        epilogue(*state["pending"])
                        state["pending"] = None
                if tail_stop is not None:
                    for s in range(subt):
                        nc.tensor.matmul(
                            ps1s[s][:, :nt],
                            lhsT=w1l_t,
                            rhs=xl_t[:, g0 + s * nt : g0 + (s + 1) * nt],
                            start=False,
                            stop=tail_stop,
                        )

            # group 0 (2048) and group 1 (1024): stream + matmul + epilogue
            infos = []
            for g in (0, 1):
                ntd = NTDS[g]
                nt = min(NT, ntd)
                subt = ntd // nt
                gs = slice(G0S[g], G0S[g] + ntd)
                xc = dma_group(g, PLANS[ntd], ntd, gs)
                if g == 0:
                    nc.sync.dma_start(out=wxl_t, in_=wxl.ap())
                ps1s = alloc_ps1(subt)
                mm_group(xc, ps1s, subt, nt, G0S[g], True, 0)
                state["pending"] = (G0S[g], ntd, subt, nt, ps1s)
                infos.append((ps1s, nt))

            # groups 2 and 3
            for g in (2, 3):
                ntd = NTDS[g]
                nt = min(NT, ntd)
                subt = ntd // nt
                gs = slice(G0S[g], G0S[g] + ntd)
                xc = dma_group(g, PLANS[ntd], ntd, gs)
                ps1s = alloc_ps1(subt)
                mm_group(xc, ps1s, subt, nt, G0S[g], True, 0)
                state["pending"] = (G0S[g], ntd, subt, nt, ps1s)

            # group 4 (the final 256): chunks 0..4 + tail mid-stream; its
            # matmul loop emits group 3's epilogue behind the stream
            g4_ntd = NTDS[4]
            g4_nt = 256
            g4_gs = slice(G0S[4], G0S[4] + g4_ntd)
            g4_xc = dma_group(4, [(0, 2), (2, 2), (4, 1)], g4_ntd, g4_gs)
            g4_ps1 = alloc_ps1(1)
            mm_group(g4_xc, g4_ps1, 1, g4_nt, G0S[4], False, 0)

            # final chunk of group 4: the last bytes of the stream
            g4_c5 = xpool.tile(
                [128, 1, g4_ntd], f16, tag="xc4_5", bufs=1, name="xc4_5"
            )
            nc.sync.dma_start(out=g4_c5, in_=xt.ap()[:, 5:6, g4_gs])
            nc.tensor.matmul(
                g4_ps1[0][:, :g4_nt],
                lhsT=w1_t[:, 5 * H : 6 * H],
                rhs=g4_c5[:, 0, 0:g4_nt],
                start=False, stop=True,
            )
            if state["pending"] is not None:
                epilogue(*state["pending"])
                state["pending"] = None
            epilogue_last(G0S[4], g4_nt, g4_ps1[0])

    nc.finalize()
    return nc


def _fold_conv_into_w1(w_conv, W1):
    """W1e[784, 100] such that x @ W1e == conv3x3(x) @ W1 (exact linear fold)."""
    W1e = np.zeros((28, 28, 100), np.float64)
    W1r = W1.astype(np.float64).reshape(26, 26, 100)
    wc = w_conv.astype(np.float64)
    for di in range(3):
        for dj in range(3):
            W1e[di : di + 26, dj : dj + 26, :] += wc[di, dj] * W1r
    return W1e.reshape(784, 100).astype(np.float32)


def kernel(x, w_conv, W1, b1, W2, b2, W3, b3):
    from concourse.bass_utils import run_bass_kernel_spmd

    global LAST_RESULTS

    x = np.asarray(x, np.float32)
    W1e = _fold_conv_into_w1(np.asarray(w_conv), np.asarray(W1))
    W1p = np.zeros((784, H), np.float32)
    W1p[:, :100] = W1e
    # wt: [128, 6*128 | W2 | W3 | b1 b2 b3] all fp16
    wt_dev = np.zeros((128, WCOLS), np.float16)
    wt_dev[:, : KC * H] = (
        W1p[: KC * 128].reshape(KC, 128, H).transpose(1, 0, 2).reshape(128, KC * H)
    ).astype(np.float16)
    wt_dev[:100, KC * H : KC * H + 100] = (
        np.asarray(W2, np.float32).astype(np.float16)
    )
    wt_dev[:100, KC * H + H : KC * H + H + O] = (
        np.asarray(W3, np.float32).astype(np.float16)
    )
    w1l_dev = W1p[KC * 128 :].astype(np.float16)      # [16, 128]
    bb_dev = np.zeros((H, 3), np.float32)
    bb_dev[:100, 0] = np.asarray(b1, np.float32)
    bb_dev[:100, 1] = np.asarray(b2, np.float32)
    bb_dev[:O, 2] = np.asarray(b3, np.float32)

    in_maps = []
    for c in range(N_CORES):
        xs = x[c * B_LOC : (c + 1) * B_LOC]          # [B_LOC, 784]
        xT = xs.T.astype(np.float16)                  # [784, B_LOC] fp16
        # main: [128, KC, B_LOC], element [p, k, n] = xT[k*128 + p, n]
        xmain = np.ascontiguousarray(
            xT[: KC * 128].reshape(KC, 128, B_LOC).transpose(1, 0, 2)
        )
        wxl_dev = np.concatenate([w1l_dev, xT[KC * 128 :]], axis=1)
        in_maps.append(
            {
                "xt": xmain,
                "wxl": np.ascontiguousarray(wxl_dev),
                "wt": wt_dev,
                "bb": bb_dev,
            }
        )

    if "nc" not in _COMPILED:
        _COMPILED["nc"] = _build_nc()
    nc = _COMPILED["nc"]

    res = run_bass_kernel_spmd(nc, in_maps, core_ids=list(range(N_CORES)))
    LAST_RESULTS = res

    out = np.empty((B, O), np.float32)
    for c in range(N_CORES):
        out[c * B_LOC : (c + 1) * B_LOC] = res.results[c]["ot"].T
    return out
